# revision 52
# baseline (speedup 1.0000x reference)
"""Trainium2 Bass kernel for nn_Correlation (FlowNet-style cost volume).

Problem: input1/input2 [8, 256, 96, 128] f32 ->
         out [8, 441, 96, 128] f32
  out[b, 21*i+j, h, w] = leaky_relu_0.1( (1/256) * sum_c
        in1[b,c,h,w] * in2pad[b,c, h+2i, w+2j] )       (pad 20 each side)

Strategy (data-parallel over B across 8 cores; per core = 1 sample):
  * Displacements are even (dilation 2): pixel (h,w) only correlates with
    in2 pixels of the same (h%2, w%2) parity class. Per parity class the
    dilated 21x21 patch is a dense 21x21 window over the 48x64 parity
    image.
  * in2 is kept as 8 resident padded parity-class images [128c, 68x84]
    bf16 in SBUF (pad 10 each side), built by large strided copies from
    chunked contiguous cast-DMA loads; the matmul ifmap reads 28x36
    windows directly via strided APs (no band duplication).
  * in1: per block-row, row-major slabs are rearranged once into
    parity-blocked lhsT tiles (one 4-dim copy per class) so matmul
    weights APs are single-free-dim.
  * Per block (8he x 16we pixels = 128 partitions): 2x2 matmuls contract
    C=256 over the 28x36 window -> PSUM band [pix, 1008]. ScalarE
    applies leaky_relu(x/C) (fused Lrelu w/ alpha) during PSUM->SBUF
    evacuation in bf16.
  * Per-pixel alignment (441 of 1008 cols, per-pixel diagonal offset) is
    done via a DRAM bounce: one contiguous write + one 3-dim diagonal
    gather read (flat DRAM-side APs make the diagonal legal). bf16 both
    ways; one DMA each.
  * PE transposes the aligned [pix, 441] (strided from the 741-wide
    gather stage) to d-major via identity matmuls; DVE copies scatter
    into a parity-interleaved bf16 out tile; gpsimd cast-DMA stores
    fp32.
  * Software-pipelined emission (write k, read k-2, transpose k-3) so
    in-order engine queues never block on in-flight DMAs.
"""

import numpy as np

import concourse.bass as bass
import concourse.mybir as mybir
from concourse.tile import TileContext
from concourse.bass_utils import run_bass_kernel_spmd
from concourse.masks import make_identity

DT = mybir.dt
AF = mybir.ActivationFunctionType

# ---- problem geometry ----
B, C, H, W = 8, 256, 96, 128
NP = 21                      # displacements per axis
ND = NP * NP                 # 441
CC = 2                       # C chunks of 128
HE, WE = H // 2, W // 2      # parity image 48 x 64
PAD = 10                     # parity-unit halo (= MAX_DISP/2)
HEP, WEP = HE + 2 * PAD, WE + 2 * PAD  # 68 x 84
CLS_F = HEP * WEP            # 5712

HB, WB = 8, 16               # pixel block (he, we); HB*WB = 128
NWB = WE // WB               # 4 w-blocks
WIN_H, WIN_W = HB + 20, WB + 20   # 28 x 36
FB = WIN_H * WIN_W           # 1008 band columns
SPAN = WIN_W * 20 + 20 + 1   # 741: per-pixel gather span
HW = H * W                   # 12288
NBLK = HE // HB              # 6 block-rows
OT_F = 4 * 2 * HB * W        # 8192: out tile free size (4 d-chunks x 16 rows)

_MAX_WAITS = 1


def _split_excess_waits(nc):
    """This walrus build accepts only ONE sync-wait per instruction; Tile
    emits multi-waits. Hoist excess waits onto same-engine NOPs inserted
    right before the over-subscribed instruction."""
    nid = 0
    for f in nc.m.functions:
        for blk in f.blocks:
            insts = list(blk.instructions)
            out = []
            changed = False
            for inst in insts:
                si = inst.sync_info
                if si is not None and si.on_wait and len(si.on_wait) > _MAX_WAITS:
                    waits = list(si.on_wait)
                    extra, keep = waits[:-_MAX_WAITS], waits[-_MAX_WAITS:]
                    for k in range(0, len(extra), _MAX_WAITS):
                        nop = mybir.InstNoOp(name=f"I-waitsplit-{nid}", ins=[], outs=[])
                        nid += 1
                        nop.engine = inst.engine
                        nop.sync_info = mybir.SyncInfo(
                            on_wait=extra[k : k + _MAX_WAITS], on_update=[]
                        )
                        out.append(nop)
                        changed = True
                    si.on_wait = keep
                    inst.sync_info = si
                out.append(inst)
            if changed:
                blk.instructions = out
    return nc


def _ap(t, off_extra, dims):
    return bass.AP(tensor=t.tensor, offset=t.offset + off_extra, ap=dims)


def _build_nc(debug=False, waitsplit=True):
    nc = bass.Bass()
    in1_d = nc.dram_tensor("in1", [C, H, W], DT.float32, kind="ExternalInput")
    in2_d = nc.dram_tensor("in2", [C, H, W], DT.float32, kind="ExternalInput")
    out_d = nc.dram_tensor("out", [ND, H, W], DT.float32, kind="ExternalOutput")
    dbg = {}
    if debug:
        dbg["band"] = nc.dram_tensor(
            "dbg_band", [128, FB], DT.bfloat16, kind="ExternalOutput"
        )
        dbg["stage"] = nc.dram_tensor(
            "dbg_stage", [128, SPAN], DT.bfloat16, kind="ExternalOutput"
        )

    with TileContext(nc) as tc:
        with (
            tc.tile_pool(name="constp", bufs=1) as constp,
            tc.tile_pool(name="slabp", bufs=2) as slabp,
            tc.tile_pool(name="lhsp", bufs=3) as lhsp,
            tc.tile_pool(name="stgp", bufs=2) as stgp,
            tc.tile_pool(name="bsbp", bufs=6) as bsbp,
            tc.tile_pool(name="stagep", bufs=5) as stagep,
            tc.tile_pool(name="aligp", bufs=4) as aligp,
            tc.tile_pool(name="relp", bufs=3) as relp,
            tc.tile_pool(name="outp", bufs=2) as outp,
            tc.tile_pool(name="psp", bufs=3, space="PSUM") as psp,
            tc.tile_pool(name="trpp", bufs=2, space="PSUM") as trpp,
            tc.tile_pool(name="dramp", bufs=8, space="DRAM") as dramp,
        ):
            identity = constp.tile([128, 128], DT.bfloat16)
            make_identity(nc, identity)
            alpha_t = constp.tile([128, 1], DT.float32, name="alpha01")
            nc.vector.memset(alpha_t[:, :], 0.1)

            cls = {}
            for cc in range(CC):
                for hp in range(2):
                    for wp in range(2):
                        cls[cc, hp, wp] = constp.tile(
                            [128, CLS_F], DT.bfloat16, name=f"cls{cc}{hp}{wp}"
                        )

            # zero the padding halos (data region is overwritten by loads)
            for ti, t in enumerate(cls.values()):
                me = nc.vector if ti % 2 == 0 else nc.gpsimd
                me.memset(_ap(t, 0, [[CLS_F, 128], [1, PAD * WEP]]), 0.0)
                me.memset(
                    _ap(t, (HEP - PAD) * WEP, [[CLS_F, 128], [1, PAD * WEP]]), 0.0
                )
                me.memset(_ap(t, PAD * WEP, [[CLS_F, 128], [WEP, HE], [1, PAD]]), 0.0)
                me.memset(
                    _ap(t, PAD * WEP + PAD + WE, [[CLS_F, 128], [WEP, HE], [1, PAD]]),
                    0.0,
                )

            def load_slab(ab):
                """in1 rows [16ab, 16ab+16) as bf16, row-major."""
                slabs = []
                for cc in range(CC):
                    s = slabp.tile([128, 2 * HB * W], DT.bfloat16, name=f"slab{cc}")
                    nc.gpsimd.dma_start(
                        s[:, :],
                        in1_d[cc * 128 : (cc + 1) * 128, 16 * ab : 16 * ab + 16, :],
                    )
                    slabs.append(s)
                return slabs

            def rearrange_lhs(ab, slabs):
                """slab -> lhs[cc,hp,wp]: [128c, wb*128 + 16he + we] (bf16).

                One 4-dim copy per class; round-robin engines."""
                # scale by 1/C here so the PE output needs no rescale
                eng = [lambda d, s: nc.vector.tensor_scalar_mul(d, s, 1.0 / C)]
                lhs = {}
                n = 0
                for cc in range(CC):
                    for hp in range(2):
                        for wp in range(2):
                            t = lhsp.tile(
                                [128, NWB * 128], DT.bfloat16, name=f"lhs{cc}{hp}{wp}"
                            )
                            src = _ap(
                                slabs[cc],
                                hp * W + wp,
                                [[2 * HB * W, 128], [2 * WB, NWB], [2 * W, HB], [2, WB]],
                            )
                            dst = _ap(
                                t,
                                0,
                                [[NWB * 128, 128], [128, NWB], [WB, HB], [1, WB]],
                            )
                            eng[0](dst, src)
                            n += 1
                            lhs[cc, hp, wp] = t
                return lhs

            slabs = load_slab(0)
            lhs_cur = rearrange_lhs(0, slabs)
            lhs_next = None

            # in2 ingest: 8 chunks of 12 full-res rows, pipelined
            prev = None

            def split_chunk(k6, stg):
                eng = [
                    lambda d, s: nc.scalar.copy(d, s),
                    lambda d, s: nc.scalar.copy(d, s),
                ]
                n = 0
                for cc in range(CC):
                    for hp in range(2):
                        for wp in range(2):
                            src = _ap(
                                stg,
                                cc * 1536 + hp * W + wp,
                                [[3072, 128], [2 * W, 6], [2, WE]],
                            )
                            dst = _ap(
                                cls[cc, hp, wp],
                                (PAD + 6 * k6) * WEP + PAD,
                                [[CLS_F, 128], [WEP, 6], [1, WE]],
                            )
                            eng[n % 2](dst, src)
                            n += 1

            def load_chunk(k6):
                stg = stgp.tile([128, 3072], DT.bfloat16, name="stg")
                for cc in range(CC):
                    nc.gpsimd.dma_start(
                        _ap(stg, cc * 1536, [[3072, 128], [1, 1536]]),
                        in2_d[cc * 128 : (cc + 1) * 128, 12 * k6 : 12 * k6 + 12, :],
                    )
                return stg

            # prologue ingests only the chunks block-row 0 needs (rows < 36);
            # chunks 3-7 are loaded/split inside the band loop so the scalar
            # queue isn't clogged ahead of the first evacuations
            chunk_stg = {}
            for k6 in range(3):
                chunk_stg[k6] = load_chunk(k6)
            for k6 in range(3):
                split_chunk(k6, chunk_stg.pop(k6))

            bands = [
                (ab, hp, wp, wb)
                for ab in range(NBLK)
                for hp in range(2)
                for wp in range(2)
                for wb in range(NWB)
            ]
            n = len(bands)
            nbr = 2 * 2 * NWB  # bands per block-row (16)
            bdrams = {}
            stages = {}
            out_t = None

            def mm(k):
                nonlocal slabs, lhs_cur, lhs_next
                ab, hp, wp, wb = bands[k]
                if k % nbr == 0 and ab + 1 < NBLK:
                    slabs = load_slab(ab + 1)
                if k % nbr == 2 and ab + 1 < NBLK:
                    lhs_next = rearrange_lhs(ab + 1, slabs)
                a = HB * ab
                # pieces at cols 0 and 512 so each stays inside one PSUM bank
                ps = psp.tile([128, 1024], DT.float32, name="ps")
                for cc in range(CC):
                    lhsT = _ap(
                        lhs_cur[cc, hp, wp],
                        wb * 128,
                        [[NWB * 128, 128], [1, 128]],
                    )
                    for t in range(2):
                        rhs = _ap(
                            cls[cc, hp, wp],
                            (a + 14 * t) * WEP + WB * wb,
                            [[CLS_F, 128], [WEP, 14], [1, WIN_W]],
                        )
                        nc.tensor.matmul(
                            ps[:, 512 * t : 512 * t + 504],
                            lhsT,
                            rhs,
                            start=(cc == 0),
                            stop=(cc == CC - 1),
                        )
                # evacuate PSUM (cast to bf16; leaky applied post-transpose)
                bsb = bsbp.tile([128, 1024], DT.bfloat16, name="bsb")
                nc.scalar.copy(bsb[:, :], ps[:, :])
                bd = dramp.tile([128, FB], DT.bfloat16, name="bd")
                nc.sync.dma_start(
                    bd[:, :],
                    _ap(bsb, 0, [[1024, 128], [512, 2], [1, 504]]),
                )
                bdrams[k] = bd
                if k % nbr == nbr - 1 and lhs_next is not None:
                    lhs_cur, lhs_next = lhs_next, None

            def rd(k):
                bd = bdrams.pop(k)
                st = stagep.tile([128, SPAN], DT.bfloat16, name="stage")
                nc.sync.dma_start(
                    st[:, :],
                    _ap(bd, 0, [[WB * FB + WIN_W, HB], [FB + 1, WB], [1, SPAN]]),
                )
                stages[k] = st
                if debug and k == 0:
                    nc.scalar.dma_start(
                        bass.AP(
                            tensor=dbg["stage"], offset=0, ap=[[SPAN, 128], [1, SPAN]]
                        ),
                        st[:, :],
                    )

            def tp(k):
                nonlocal out_t
                ab, hp, wp, wb = bands[k]
                if k % nbr == 0:
                    out_t = outp.tile([128, OT_F], DT.bfloat16, name="ot")
                st = stages.pop(k)
                # unfold: alig[p, 21i+j] = stage[p, 36i+j] (dense 441 cols)
                alig = aligp.tile([128, ND], DT.bfloat16, name="alig")
                ueng = (
                    (lambda d, s: nc.scalar.copy(d, s))
                    if k % 2 == 0
                    else (lambda d, s: nc.gpsimd.tensor_copy(d, s))
                )
                ueng(
                    _ap(alig, 0, [[ND, 128], [NP, NP], [1, NP]]),
                    _ap(st, 0, [[SPAN, 128], [WIN_W, NP], [1, NP]]),
                )
                tr = trpp.tile([128, 512], DT.float32, name="tr")
                for dc in range(4):
                    nd = min(128, ND - 128 * dc)
                    nc.tensor.matmul(
                        tr[0:nd, 128 * dc : 128 * dc + 128],
                        alig[:, 128 * dc : 128 * dc + nd],
                        identity[:, :],
                        start=True,
                        stop=True,
                    )
                # leaky: out = 0.1*x + max(0.9*x, 0) computed as relu-part (DVE
                # tensor_scalar) + stt combine during the out_t scatter
                rel = relp.tile([128, 512], DT.bfloat16, name="rel")
                nc.vector.tensor_scalar(
                    rel[:, :],
                    tr[:, :],
                    0.9,
                    0.0,
                    mybir.AluOpType.mult,
                    mybir.AluOpType.max,
                )
                base = hp * W + 2 * WB * wb + wp
                nc.vector.scalar_tensor_tensor(
                    _ap(
                        out_t,
                        base,
                        [[OT_F, 128], [2 * HB * W, 3], [2 * W, HB], [2, WB]],
                    ),
                    _ap(tr, 0, [[512, 128], [128, 3], [WB, HB], [1, WB]]),
                    0.1,
                    _ap(rel, 0, [[512, 128], [128, 3], [WB, HB], [1, WB]]),
                    mybir.AluOpType.mult,
                    mybir.AluOpType.add,
                )
                nc.vector.scalar_tensor_tensor(
                    _ap(
                        out_t,
                        3 * 2 * HB * W + base,
                        [[OT_F, 57], [2 * W, HB], [2, WB]],
                    ),
                    _ap(tr, 3 * 128, [[512, 57], [WB, HB], [1, WB]]),
                    0.1,
                    _ap(rel, 3 * 128, [[512, 57], [WB, HB], [1, WB]]),
                    mybir.AluOpType.mult,
                    mybir.AluOpType.add,
                )
                if k % nbr == nbr - 1:
                    # queue the store in 4 pieces, spread over the next
                    # block-row's bands so the 3.6MB burst doesn't starve the
                    # bounce DMAs (evac stalls at block-row cadence otherwise)
                    for j in range(4):
                        pending_stores.append((ab, out_t, j))

            def store_piece(ab, ot, j):
                nc.gpsimd.dma_start(
                    bass.AP(
                        tensor=out_d,
                        offset=32 * j * HW + 2 * HB * ab * W,
                        ap=[[HW, 32], [128 * HW, 3], [1, 2 * HB * W]],
                    ),
                    _ap(
                        ot,
                        32 * j * OT_F,
                        [[OT_F, 32], [2 * HB * W, 3], [1, 2 * HB * W]],
                    ),
                )
                if j == 3:
                    nc.gpsimd.dma_start(
                        bass.AP(
                            tensor=out_d,
                            offset=384 * HW + 2 * HB * ab * W,
                            ap=[[HW, 57], [1, 2 * HB * W]],
                        ),
                        _ap(ot, 3 * 2 * HB * W, [[OT_F, 57], [1, 2 * HB * W]]),
                    )

            pending_stores = []
            for k in range(n + 4):
                if k < n:
                    # stream the remaining in2 chunks just-in-time (load at
                    # k=4(c-3)+1, split 2 bands later) so the scalar queue
                    # isn't clogged ahead of the first evacuations
                    if k % 4 == 1 and 3 + k // 4 < 8:
                        chunk_stg[3 + k // 4] = load_chunk(3 + k // 4)
                    if k % 4 == 3 and 3 + k // 4 in chunk_stg:
                        split_chunk(3 + k // 4, chunk_stg.pop(3 + k // 4))
                    mm(k)
                # read one band deeper than strictly needed so the in-order
                # sync queue rarely blocks on an in-flight write; transpose
                # stays at k-4 (deeper hurt: see v12 post-mortem)
                if 0 <= k - 3 < n:
                    rd(k - 3)
                if 0 <= k - 4 < n:
                    tp(k - 4)
                if pending_stores and k % 4 == 3:
                    store_piece(*pending_stores.pop(0))
            while pending_stores:
                store_piece(*pending_stores.pop(0))

    if waitsplit:
        _split_excess_waits(nc)
    return nc


_NC_CACHE = None


def _get_nc():
    global _NC_CACHE
    if _NC_CACHE is None:
        _NC_CACHE = _build_nc()
    return _NC_CACHE


def kernel(input1, input2):
    input1 = np.ascontiguousarray(np.asarray(input1, dtype=np.float32))
    input2 = np.ascontiguousarray(np.asarray(input2, dtype=np.float32))
    assert input1.shape == (B, C, H, W) and input2.shape == (B, C, H, W)
    nc = _get_nc()
    in_maps = [{"in1": input1[b], "in2": input2[b]} for b in range(B)]
    res = run_bass_kernel_spmd(nc, in_maps, core_ids=list(range(B)))
    return np.stack([res.results[b]["out"] for b in range(B)], axis=0)
